# revision 4
# baseline (speedup 1.0000x reference)
"""Trainium2 Bass kernel for nn_SamplePolicy_14886356648064.

Reference semantics (T=4 resample rounds, K=4 vote threshold, H=8 heads):
  each round: per-head argmax over src -> presence vector per head ->
  counting = sum of presence over heads -> trigger = counting.max() <= K ->
  if trigger, replace all heads with head `sampled_t` (broadcast).

Exact algebraic collapse of the T-loop (see baseline notes): only round 0's
trigger and sampled_0 matter, for ANY input -> output is either the input
(no trigger) or broadcast(aw[3]).

Device work: per-row block maxima of the fp16-cast input, one head per core.
fp16 rounding is monotone (x <= y => f16(x) <= f16(y)), so the f32 row
argmax always lives in a block whose fp16 block-max ties the row's fp16 max.
The device streams the 16MB fp16 head slice once (DMA-bound, ~430 GB/s) and
reduces every 128-wide block to its max on the vector engine; the host then
scans only the tied candidate blocks (~2 of 32 per row for uniform data) in
f32 for the exact first-occurrence argmax, and runs the tiny vote logic.

Layout: head slice [2048, 4096] fp16 viewed as x[tile, p, k] = [8, 128, 8192]
(row = tile*256 + 2p + t, k = t*4096 + c): each partition holds 2 consecutive
rows -> 16KB contiguous DMA descriptors (same descriptor shape that measured
430 GB/s aggregate in f32). Reduces are split into <=16-block instructions
(32-block reduces fall off a perf cliff: 1.24 vs 0.52 ns/elem measured).

sampled_0 = jax.random.randint(jax.random.fold_in(jax.random.key(42), 0),
                               (), 0, 7) == 3 (threefry, platform independent).
"""

import numpy as np

H = 8
TGT = 2048
SRC = 4096
P = 128            # SBUF partitions per tile
RPP = 2            # rows per partition
W = RPP * SRC      # 8192 elements per partition per row-tile
NT = TGT // (P * RPP)  # 8 row-tiles
NBLK = 32          # block maxima per original row
BLK = SRC // NBLK  # 128
KPB = W // BLK     # 64 block maxima per partition per tile
K_THRESH = 4
SAMPLED_T0 = 3
BUFS = 8
RED = 2048         # max columns per reduce instruction (16 blocks)

# DMA jobs: (row_tile, col_start, col_len) in the [NT, P, W] fp16 layout.
# Full tiles are 2MB contiguous loads; the last two tiles taper so the
# vector-engine reduce train drains alongside the DMA stream.
JOBS = [(i, 0, W) for i in range(NT - 2)]
JOBS += [(NT - 2, 0, W // 2), (NT - 2, W // 2, W // 2)]
JOBS += [(NT - 1, 0, W // 2)]
JOBS += [(NT - 1, W // 2, W // 4)]
JOBS += [(NT - 1, 3 * W // 4, W // 8), (NT - 1, 7 * W // 8, W // 8)]
NJOBS = len(JOBS)
BULK_JOBS = NJOBS - 4  # jobs covering tiles 0..NT-2

_cache = {}


def _build_nc():
    """Raw Bass program, one head per core.

    Pipeline: the two HWDGE rings (sync + scalar engines) alternately stream
    [128, col] fp16 chunks from DRAM into a BUFS-deep SBUF ring; the vector
    engine reduces each chunk to its block maxima (<=16 blocks per reduce
    instruction) accumulated in SBUF; gpsimd issues the bulk block-max store
    (overlapping the last tile's work) and the vector engine issues the final
    tile's store directly after its last reduce (no semaphore hop).
    """
    from contextlib import ExitStack

    import concourse.bass as bass
    import concourse.mybir as mybir

    nc = bass.Bass()
    f16 = mybir.dt.float16
    x = nc.declare_dram_parameter("x", [NT, P, W], f16, isOutput=False)
    # bm[p, i, t*NBLK + b] = max of block b of row i*256 + 2p + t
    bm = nc.declare_dram_parameter("bm", [P, NT, KPB], f16, isOutput=True)

    with ExitStack() as ctx:
        tiles = ctx.enter_context(nc.sbuf_tensor([P, BUFS, W], f16))
        bmsb = ctx.enter_context(nc.sbuf_tensor([P, NT, KPB], f16))
        # one load-completion semaphore per ring slot: increments to a slot's
        # semaphore are strictly ordered by the WAR chain, so thresholds are
        # race-free under any cross-queue DMA completion order
        s_in = [ctx.enter_context(nc.semaphore(f"s_in{j}")) for j in range(BUFS)]
        s_red = ctx.enter_context(nc.semaphore("s_red"))
        s_out = ctx.enter_context(nc.semaphore("s_out"))
        block = ctx.enter_context(nc.Block())

        def issue_loads(eng, parity):
            for j, (i, c0, clen) in enumerate(JOBS):
                if j % 2 != parity:
                    continue
                if j >= BUFS:
                    # WAR: don't overwrite slot until its reduce finished
                    eng.wait_ge(s_red, j - BUFS + 1)
                eng.dma_start(
                    out=tiles[:, j % BUFS, :clen],
                    in_=x[i, :, c0 : c0 + clen],
                ).then_inc(s_in[j % BUFS], 16)

        @block.sync
        def _(sync):
            issue_loads(sync, 0)
            sync.wait_ge(s_out, 32)

        @block.scalar
        def _(scalar):
            issue_loads(scalar, 1)

        @block.gpsimd
        def _(gpsimd):
            # bulk store overlaps the final row-tile's loads/reduces
            gpsimd.wait_ge(s_red, BULK_JOBS)
            gpsimd.dma_start(
                out=bm[:, : NT - 1, :], in_=bmsb[:, : NT - 1, :]
            ).then_inc(s_out, 16)
            gpsimd.wait_ge(s_red, NJOBS)
            gpsimd.dma_start(
                out=bm[:, NT - 1, :], in_=bmsb[:, NT - 1, :]
            ).then_inc(s_out, 16)

        @block.vector
        def _(vector):
            for j, (i, c0, clen) in enumerate(JOBS):
                vector.wait_ge(s_in[j % BUFS], 16 * (j // BUFS + 1))
                for s0 in range(c0, c0 + clen, RED):
                    sl = min(RED, c0 + clen - s0)
                    r = nc.vector.reduce_max(
                        out=bmsb[:, i, s0 // BLK : (s0 + sl) // BLK],
                        in_=tiles[:, j % BUFS, s0 - c0 : s0 - c0 + sl].rearrange(
                            "p (b c) -> p b c", c=BLK
                        ),
                        axis=mybir.AxisListType.X,
                    )
                r.then_inc(s_red, 1)

    return nc


def _get_nc():
    if "nc" not in _cache:
        _cache["nc"] = _build_nc()
    return _cache["nc"]


def run_device(aw, **run_kwargs):
    """Run the per-head fp16 block-max kernel on 8 cores.

    Takes the full f32 [H, TGT, SRC] tensor; returns ([H, TGT, NBLK] fp16
    block maxima of the fp16-cast data, results).
    """
    from concourse.bass_utils import run_bass_kernel_spmd

    nc = _get_nc()
    aw16 = aw.astype(np.float16)
    in_maps = [
        {"x": np.ascontiguousarray(aw16[c]).reshape(NT, P, W)} for c in range(H)
    ]
    res = run_bass_kernel_spmd(nc, in_maps, list(range(H)), **run_kwargs)
    # bm[p, i, t*NBLK+b] -> row-major [TGT, NBLK]: row = i*256 + 2p + t
    bms = []
    for c in range(H):
        r = res.results[c]["bm"]  # [P, NT, KPB]
        bms.append(
            r.transpose(1, 0, 2).reshape(NT, P, RPP, NBLK).reshape(TGT, NBLK)
        )
    return np.stack(bms), res


def _exact_argmax(aw, bm):
    """Exact first-occurrence np.argmax(aw, -1) from fp16 block maxima.

    fp16 rounding is monotone, so every element equal to the f32 row max
    lives in a block whose fp16 block max ties the row's fp16 max. Scanning
    the tied blocks in ascending order preserves first-occurrence order.
    """
    rowmax = bm.max(-1, keepdims=True)
    mask = bm == rowmax  # [H, TGT, NBLK] candidate blocks
    cmax = int(mask.sum(-1).max())
    # candidate block indices in ascending order, padded with non-candidates
    order = np.argsort(~mask, axis=-1, kind="stable")[..., :cmax]
    valid = np.take_along_axis(mask, order, -1)
    blocks = aw.reshape(H, TGT, NBLK, BLK)
    win = np.take_along_axis(blocks, order[..., None], axis=2)  # [H,T,cmax,BLK]
    win = np.where(valid[..., None], win, -np.inf).reshape(H, TGT, cmax * BLK)
    j = win.argmax(-1)
    b = np.take_along_axis(order, (j // BLK)[..., None], -1)[..., 0]
    return b * BLK + j % BLK


def kernel(attention_weight):
    aw = np.asarray(attention_weight)
    assert aw.shape == (H, TGT, SRC), aw.shape
    aw = aw.astype(np.float32, copy=False)

    try:
        bm, _ = run_device(aw)
    except Exception as e:  # device path failed: fall back to host blockmax
        import traceback

        traceback.print_exc()
        print(f"WARNING: device path failed ({e!r}); falling back to numpy")
        bm = aw.astype(np.float16).reshape(H, TGT, NBLK, BLK).max(-1)

    cand = _exact_argmax(aw, bm)  # [H, TGT]
    present = np.zeros((H, SRC), np.float32)
    present[np.arange(H)[:, None], cand] = 1.0
    counting = present.sum(axis=0)

    if counting.max() <= K_THRESH:
        return np.broadcast_to(aw[SAMPLED_T0], aw.shape).copy()
    return aw


# revision 5
# speedup vs baseline: 1.2497x; 1.2497x over previous
"""Trainium2 Bass kernel for nn_SamplePolicy_14886356648064.

Reference semantics (T=4 resample rounds, K=4 vote threshold, H=8 heads):
  each round: per-head argmax over src -> presence vector per head ->
  counting = sum of presence over heads -> trigger = counting.max() <= K ->
  if trigger, replace all heads with head `sampled_t` (broadcast).

Exact algebraic collapse of the T-loop (see baseline notes): only round 0's
trigger and sampled_0 matter, for ANY input -> output is either the input
(no trigger) or broadcast(aw[3]).

Device work: per-row block maxima of the fp16-cast input, one head per core.
fp16 rounding is monotone (x <= y => f16(x) <= f16(y)), so the f32 row
argmax always lives in a block whose fp16 block-max ties the row's fp16 max.
The device streams the 16MB fp16 head slice once (~430 GB/s aggregate over
two HWDGE rings) and reduces every 128-wide block to its max on the vector
engine; the host then scans only the tied candidate blocks (~2 of 32 per row
for uniform data) in f32 for the exact first-occurrence argmax, and runs the
tiny vote logic.

Measured DVE cost model (HW, 128 partitions):
  tensor_reduce: 1.04 ns/input-elem + 150ns, any dtype (no fast mode).
  tensor_tensor (max): 0.52 ns/output-elem + 150ns for packed 2-byte dtypes
  (mode 2x_1p); f32 gets 1.04.
So block maxima are computed as a fold tree (pairwise tensor_max halving the
block width) finished by a small tensor_reduce: 5.28us per 2MB chunk vs
9.1us for direct 16-block reduces. Pool/scalar engines cannot run tensor ops
on TRN2 (walrus engine check) - the DVE does all folding.

Schedule: the two rings are STAGGERED (different job sizes) so chunk
completions interleave every ~2.4-4.9us instead of arriving in 4MB pairs;
the vector engine consumes chunks in predicted arrival order. The full 16MB
head slice fits in SBUF (128KB/partition), so every job owns a dedicated
SBUF region - no write-after-read hazards, no slot-reuse waits.

Layout: head slice [2048, 4096] fp16 viewed as x[tile, p, k] = [8, 128, 8192]
(row = tile*256 + 2p + t, k = t*4096 + c): each partition holds 2 consecutive
rows -> 16KB contiguous DMA descriptors at full-width jobs.

sampled_0 = jax.random.randint(jax.random.fold_in(jax.random.key(42), 0),
                               (), 0, 7) == 3 (threefry, platform independent).
"""

import numpy as np

H = 8
TGT = 2048
SRC = 4096
P = 128            # SBUF partitions
RPP = 2            # rows per partition
W = RPP * SRC      # 8192 elements per partition per row-tile
NT = TGT // (P * RPP)  # 8 row-tiles
NBLK = 32          # block maxima per original row
BLK = SRC // NBLK  # 128
KPB = W // BLK     # 64 block maxima per partition per tile
K_THRESH = 4
SAMPLED_T0 = 3

# (ring, clen) in VECTOR (predicted-arrival) order; ring A first job is 4096
# and ring B 2048 so completions stagger. Chunks chop [0..8*W) contiguously
# and never straddle a row-tile.
SCHEDULE = [
    ("B", 2048), ("A", 4096), ("A", 2048), ("B", 4096),
    ("A", 4096), ("B", 4096), ("B", 4096), ("A", 8192),
    ("B", 8192), ("A", 8192), ("B", 8192), ("A", 8192),
]
# fold depth per chunk size (then one reduce over the remaining width)
FOLDK = {8192: 4, 4096: 3, 2048: 2, 1024: 1, 512: 1}

JOBS = []          # (ring, tile, c0, clen, g0) in vector order
_g = 0
for _ring, _clen in SCHEDULE:
    assert _g // W == (_g + _clen - 1) // W, "chunk straddles a tile"
    JOBS.append((_ring, _g // W, _g % W, _clen, _g))
    _g += _clen
assert _g == NT * W
NJOBS = len(JOBS)
# number of jobs (in vector order) covering tiles 0..NT-2
BULK_JOBS = sum(1 for j in JOBS if j[1] < NT - 1)

_cache = {}


def _build_nc():
    """Raw Bass program, one head per core."""
    from contextlib import ExitStack

    import concourse.bass as bass
    import concourse.mybir as mybir

    nc = bass.Bass()
    f16 = mybir.dt.float16
    x = nc.declare_dram_parameter("x", [NT, P, W], f16, isOutput=False)
    # bm[p, i, t*NBLK + b] = max of block b of row i*256 + 2p + t
    bm = nc.declare_dram_parameter("bm", [P, NT, KPB], f16, isOutput=True)

    with ExitStack() as ctx:
        tiles = ctx.enter_context(nc.sbuf_tensor([P, NT * W], f16))
        scra = ctx.enter_context(nc.sbuf_tensor([P, W // 2], f16))
        scrb = ctx.enter_context(nc.sbuf_tensor([P, W // 4], f16))
        bmsb = ctx.enter_context(nc.sbuf_tensor([P, NT, KPB], f16))
        s_in = [ctx.enter_context(nc.semaphore(f"s_in{j}")) for j in range(NJOBS)]
        s_red = ctx.enter_context(nc.semaphore("s_red"))
        s_out = ctx.enter_context(nc.semaphore("s_out"))
        block = ctx.enter_context(nc.Block())

        def issue_loads(eng, ring):
            for j, (r, i, c0, clen, g0) in enumerate(JOBS):
                if r != ring:
                    continue
                eng.dma_start(
                    out=tiles[:, g0 : g0 + clen],
                    in_=x[i, :, c0 : c0 + clen],
                ).then_inc(s_in[j], 16)

        @block.sync
        def _(sync):
            issue_loads(sync, "A")
            sync.wait_ge(s_out, 32)

        @block.scalar
        def _(scalar):
            issue_loads(scalar, "B")

        @block.gpsimd
        def _(gpsimd):
            # bulk store overlaps the final row-tile's fold train
            gpsimd.wait_ge(s_red, BULK_JOBS)
            gpsimd.dma_start(
                out=bm[:, : NT - 1, :], in_=bmsb[:, : NT - 1, :]
            ).then_inc(s_out, 16)
            gpsimd.wait_ge(s_red, NJOBS)
            gpsimd.dma_start(
                out=bm[:, NT - 1, :], in_=bmsb[:, NT - 1, :]
            ).then_inc(s_out, 16)

        @block.vector
        def _(vector):
            for j, (r, i, c0, clen, g0) in enumerate(JOBS):
                vector.wait_ge(s_in[j], 16)
                nblk = clen // BLK
                k = FOLDK[clen]
                # fold tree: halve block width k times, ping-pong scratch
                src = tiles[:, g0 : g0 + clen]
                w = BLK
                for step in range(k):
                    dst = (scra if step % 2 == 0 else scrb)[:, : nblk * w // 2]
                    sv = src.rearrange("p (b c) -> p b c", c=w)
                    nc.vector.tensor_max(
                        out=dst.rearrange("p (b c) -> p b c", c=w // 2),
                        in0=sv[:, :, : w // 2],
                        in1=sv[:, :, w // 2 :],
                    )
                    src, w = dst, w // 2
                # finish with one reduce over the remaining width
                nc.vector.reduce_max(
                    out=bmsb[:, i, c0 // BLK : (c0 + clen) // BLK],
                    in_=src.rearrange("p (b c) -> p b c", c=w),
                    axis=mybir.AxisListType.X,
                ).then_inc(s_red, 1)

    return nc


def _get_nc():
    if "nc" not in _cache:
        _cache["nc"] = _build_nc()
    return _cache["nc"]


def run_device(aw, **run_kwargs):
    """Run the per-head fp16 block-max kernel on 8 cores.

    Takes the full f32 [H, TGT, SRC] tensor; returns ([H, TGT, NBLK] fp16
    block maxima of the fp16-cast data, results).
    """
    from concourse.bass_utils import run_bass_kernel_spmd

    nc = _get_nc()
    aw16 = aw.astype(np.float16)
    in_maps = [
        {"x": np.ascontiguousarray(aw16[c]).reshape(NT, P, W)} for c in range(H)
    ]
    res = run_bass_kernel_spmd(nc, in_maps, list(range(H)), **run_kwargs)
    # bm[p, i, t*NBLK+b] -> row-major [TGT, NBLK]: row = i*256 + 2p + t
    bms = []
    for c in range(H):
        r = res.results[c]["bm"]  # [P, NT, KPB]
        bms.append(
            r.transpose(1, 0, 2).reshape(NT, P, RPP, NBLK).reshape(TGT, NBLK)
        )
    return np.stack(bms), res


def _exact_argmax(aw, bm):
    """Exact first-occurrence np.argmax(aw, -1) from fp16 block maxima.

    fp16 rounding is monotone, so every element equal to the f32 row max
    lives in a block whose fp16 block max ties the row's fp16 max. Scanning
    the tied blocks in ascending order preserves first-occurrence order.
    """
    rowmax = bm.max(-1, keepdims=True)
    mask = bm == rowmax  # [H, TGT, NBLK] candidate blocks
    cmax = int(mask.sum(-1).max())
    # candidate block indices in ascending order, padded with non-candidates
    order = np.argsort(~mask, axis=-1, kind="stable")[..., :cmax]
    valid = np.take_along_axis(mask, order, -1)
    blocks = aw.reshape(H, TGT, NBLK, BLK)
    win = np.take_along_axis(blocks, order[..., None], axis=2)  # [H,T,cmax,BLK]
    win = np.where(valid[..., None], win, -np.inf).reshape(H, TGT, cmax * BLK)
    j = win.argmax(-1)
    b = np.take_along_axis(order, (j // BLK)[..., None], -1)[..., 0]
    return b * BLK + j % BLK


def kernel(attention_weight):
    aw = np.asarray(attention_weight)
    assert aw.shape == (H, TGT, SRC), aw.shape
    aw = aw.astype(np.float32, copy=False)

    try:
        bm, _ = run_device(aw)
    except Exception as e:  # device path failed: fall back to host blockmax
        import traceback

        traceback.print_exc()
        print(f"WARNING: device path failed ({e!r}); falling back to numpy")
        bm = aw.astype(np.float16).reshape(H, TGT, NBLK, BLK).max(-1)

    cand = _exact_argmax(aw, bm)  # [H, TGT]
    present = np.zeros((H, SRC), np.float32)
    present[np.arange(H)[:, None], cand] = 1.0
    counting = present.sum(axis=0)

    if counting.max() <= K_THRESH:
        return np.broadcast_to(aw[SAMPLED_T0], aw.shape).copy()
    return aw


# revision 6
# speedup vs baseline: 1.2935x; 1.0351x over previous
"""Trainium2 Bass kernel for nn_SamplePolicy_14886356648064.

Reference semantics (T=4 resample rounds, K=4 vote threshold, H=8 heads):
  each round: per-head argmax over src -> presence vector per head ->
  counting = sum of presence over heads -> trigger = counting.max() <= K ->
  if trigger, replace all heads with head `sampled_t` (broadcast).

Exact algebraic collapse of the T-loop: only round 0's trigger and sampled_0
matter, for ANY input -> output is either the input (no trigger) or
broadcast(aw[3]).  sampled_0 = jax.random.randint(fold_in(key(42), 0), (),
0, 7) == 3 (threefry, platform independent).

Device work: per-row 128-wide block maxima of the fp16-cast input, one head
per core. fp16 rounding is monotone (x <= y => f16(x) <= f16(y)), so the f32
row argmax always lives in a block whose fp16 block-max ties the row's fp16
max. The device streams the 16MB fp16 head slice once; the host then scans
only the tied candidate blocks (~2 of 32 per row for uniform data) in f32
for the exact first-occurrence argmax, and runs the tiny vote logic.

Measured HW model driving the design (all 128 partitions in parallel):
  DMA: two HWDGE rings, ~425 GB/s aggregate when both active (sync ring
    ~236, scalar ring ~189); 16 DMA engines at ~26.8 GB/s each, 100% busy.
  DVE tensor_reduce: 1.04 ns/input-elem + 150ns, ANY dtype (no fast mode).
  DVE tensor_tensor max: 0.52 ns/output-elem + 150ns for packed 2-byte
    dtypes (2x_1p mode); pool/scalar engines cannot run tensor ops on TRN2.
Block maxima = fold tree (pairwise tensor_max halving block width 4x) + one
small reduce: 5.3us per 2MB chunk vs 9.1us for direct reduces.

Layout: head slice [2048, 4096] fp16 viewed as x[p, k] = [128, 65536]
(partition p holds rows 16p..16p+15 contiguously; k = t*4096 + c). Chunks
are arbitrary [c0, c0+clen) column ranges - no tile boundaries - and the
block-max output bm[p, t*32+b] reshapes straight to row-major [2048, 32].
The full 16MB fits in SBUF (128KB/partition), so every DMA job owns a
dedicated region: no write-after-read hazards anywhere.

Schedule: ring sizes/order chosen by an offline search against the
calibrated two-ring rate model so chunk completions (ring-FIFO) match the
vector engine's consumption order with minimal starvation under +-10% ring
rate skew. Small chunks first (vector starts ~10.5us), one big final chunk
(vector is backlogged at the end anyway; big chunks cost least per element).
"""

import numpy as np

H = 8
TGT = 2048
SRC = 4096
P = 128            # SBUF partitions
RPP = 16           # rows per partition
W = RPP * SRC      # 65536 elements per partition
NBLK = 32          # block maxima per original row
BLK = SRC // NBLK  # 128
NBLK_ALL = W // BLK  # 512 block maxima per partition
K_THRESH = 4
SAMPLED_T0 = 3

# (ring, clen) in VECTOR (predicted-arrival) order; c0 = running sum.
SCHEDULE = [
    ("A", 2048), ("A", 1024), ("A", 1024), ("A", 4096),
    ("B", 8192), ("A", 4096), ("A", 4096), ("A", 2048),
    ("A", 2048), ("B", 8192), ("A", 4096), ("B", 8192),
    ("A", 16384),
]
# fold depth per chunk size (then one reduce over the remaining width)
FOLDK = {16384: 4, 8192: 4, 4096: 3, 2048: 2, 1024: 1}

JOBS = []          # (ring, c0, clen) in vector order
_g = 0
for _ring, _clen in SCHEDULE:
    JOBS.append((_ring, _g, _clen))
    _g += _clen
assert _g == W
NJOBS = len(JOBS)
FINAL_C0 = JOBS[-1][1]          # bulk store covers blocks [0, FINAL_C0/BLK)

_cache = {}


def _build_nc():
    """Raw Bass program, one head per core."""
    from contextlib import ExitStack

    import concourse.bass as bass
    import concourse.mybir as mybir

    nc = bass.Bass()
    f16 = mybir.dt.float16
    x = nc.declare_dram_parameter("x", [P, W], f16, isOutput=False)
    # bm[p, t*NBLK + b] = max of block b of row 16p + t
    bm = nc.declare_dram_parameter("bm", [P, NBLK_ALL], f16, isOutput=True)

    with ExitStack() as ctx:
        tiles = ctx.enter_context(nc.sbuf_tensor([P, W], f16))
        scra = ctx.enter_context(nc.sbuf_tensor([P, 8192], f16))
        scrb = ctx.enter_context(nc.sbuf_tensor([P, 4096], f16))
        bmsb = ctx.enter_context(nc.sbuf_tensor([P, NBLK_ALL], f16))
        s_in = [ctx.enter_context(nc.semaphore(f"s_in{j}")) for j in range(NJOBS)]
        s_red = ctx.enter_context(nc.semaphore("s_red"))
        s_out = ctx.enter_context(nc.semaphore("s_out"))
        block = ctx.enter_context(nc.Block())

        def issue_loads(eng, ring):
            for j, (r, c0, clen) in enumerate(JOBS):
                if r != ring:
                    continue
                eng.dma_start(
                    out=tiles[:, c0 : c0 + clen],
                    in_=x[:, c0 : c0 + clen],
                ).then_inc(s_in[j], 16)

        @block.sync
        def _(sync):
            issue_loads(sync, "A")
            sync.wait_ge(s_out, 32)

        @block.scalar
        def _(scalar):
            issue_loads(scalar, "B")

        @block.gpsimd
        def _(gpsimd):
            # bulk store overlaps the final chunk's fold train
            gpsimd.wait_ge(s_red, NJOBS - 1)
            gpsimd.dma_start(
                out=bm[:, : FINAL_C0 // BLK], in_=bmsb[:, : FINAL_C0 // BLK]
            ).then_inc(s_out, 16)
            gpsimd.wait_ge(s_red, NJOBS)
            gpsimd.dma_start(
                out=bm[:, FINAL_C0 // BLK :], in_=bmsb[:, FINAL_C0 // BLK :]
            ).then_inc(s_out, 16)

        @block.vector
        def _(vector):
            for j, (r, c0, clen) in enumerate(JOBS):
                vector.wait_ge(s_in[j], 16)
                nblk = clen // BLK
                # fold tree: halve block width FOLDK times, ping-pong scratch
                src = tiles[:, c0 : c0 + clen]
                w = BLK
                for step in range(FOLDK[clen]):
                    dst = (scra if step % 2 == 0 else scrb)[:, : nblk * w // 2]
                    sv = src.rearrange("p (b c) -> p b c", c=w)
                    nc.vector.tensor_max(
                        out=dst.rearrange("p (b c) -> p b c", c=w // 2),
                        in0=sv[:, :, : w // 2],
                        in1=sv[:, :, w // 2 :],
                    )
                    src, w = dst, w // 2
                # finish with one reduce over the remaining width
                nc.vector.reduce_max(
                    out=bmsb[:, c0 // BLK : (c0 + clen) // BLK],
                    in_=src.rearrange("p (b c) -> p b c", c=w),
                    axis=mybir.AxisListType.X,
                ).then_inc(s_red, 1)

    return nc


def _get_nc():
    if "nc" not in _cache:
        _cache["nc"] = _build_nc()
    return _cache["nc"]


def run_device(aw, **run_kwargs):
    """Run the per-head fp16 block-max kernel on 8 cores.

    Takes the full f32 [H, TGT, SRC] tensor; returns ([H, TGT, NBLK] fp16
    block maxima of the fp16-cast data, results).
    """
    from concourse.bass_utils import run_bass_kernel_spmd

    nc = _get_nc()
    aw16 = aw.astype(np.float16)
    in_maps = [
        {"x": np.ascontiguousarray(aw16[c]).reshape(P, W)} for c in range(H)
    ]
    res = run_bass_kernel_spmd(nc, in_maps, list(range(H)), **run_kwargs)
    # bm[p, t*NBLK+b] -> row-major [TGT, NBLK]: row = 16p + t
    bms = [res.results[c]["bm"].reshape(TGT, NBLK) for c in range(H)]
    return np.stack(bms), res


def _exact_argmax(aw, bm):
    """Exact first-occurrence np.argmax(aw, -1) from fp16 block maxima.

    fp16 rounding is monotone, so every element equal to the f32 row max
    lives in a block whose fp16 block max ties the row's fp16 max. Scanning
    the tied blocks in ascending order preserves first-occurrence order.
    """
    rowmax = bm.max(-1, keepdims=True)
    mask = bm == rowmax  # [H, TGT, NBLK] candidate blocks
    cmax = int(mask.sum(-1).max())
    # candidate block indices in ascending order, padded with non-candidates
    order = np.argsort(~mask, axis=-1, kind="stable")[..., :cmax]
    valid = np.take_along_axis(mask, order, -1)
    blocks = aw.reshape(H, TGT, NBLK, BLK)
    win = np.take_along_axis(blocks, order[..., None], axis=2)  # [H,T,cmax,BLK]
    win = np.where(valid[..., None], win, -np.inf).reshape(H, TGT, cmax * BLK)
    j = win.argmax(-1)
    b = np.take_along_axis(order, (j // BLK)[..., None], -1)[..., 0]
    return b * BLK + j % BLK


def kernel(attention_weight):
    aw = np.asarray(attention_weight)
    assert aw.shape == (H, TGT, SRC), aw.shape
    aw = aw.astype(np.float32, copy=False)

    try:
        bm, _ = run_device(aw)
    except Exception as e:  # device path failed: fall back to host blockmax
        import traceback

        traceback.print_exc()
        print(f"WARNING: device path failed ({e!r}); falling back to numpy")
        bm = aw.astype(np.float16).reshape(H, TGT, NBLK, BLK).max(-1)

    cand = _exact_argmax(aw, bm)  # [H, TGT]
    present = np.zeros((H, SRC), np.float32)
    present[np.arange(H)[:, None], cand] = 1.0
    counting = present.sum(axis=0)

    if counting.max() <= K_THRESH:
        return np.broadcast_to(aw[SAMPLED_T0], aw.shape).copy()
    return aw


# revision 8
# speedup vs baseline: 1.3645x; 1.0549x over previous
"""Trainium2 Bass kernel for nn_SamplePolicy_14886356648064.

Reference semantics (T=4 resample rounds, K=4 vote threshold, H=8 heads):
  each round: per-head argmax over src -> presence vector per head ->
  counting = sum of presence over heads -> trigger = counting.max() <= K ->
  if trigger, replace all heads with head `sampled_t` (broadcast).

Exact algebraic collapse of the T-loop: only round 0's trigger and sampled_0
matter, for ANY input -> output is either the input (no trigger) or
broadcast(aw[3]).  sampled_0 = jax.random.randint(fold_in(key(42), 0), (),
0, 7) == 3 (threefry, platform independent).

Device work: per-row 128-wide block maxima of the fp16-cast input, one head
per core. fp16 rounding is monotone (x <= y => f16(x) <= f16(y)), so the f32
row argmax always lives in a block whose fp16 block-max ties the row's fp16
max. The device streams the 16MB fp16 head slice once; the host then scans
only the tied candidate blocks (~2 of 32 per row for uniform data) in f32
for the exact first-occurrence argmax, and runs the tiny vote logic.

Measured HW model driving the design (all 128 partitions in parallel):
  DMA: two HWDGE rings, ~425 GB/s aggregate when both active (sync ring
    ~236, scalar ring ~189); 16 DMA engines at ~26.8 GB/s each, 100% busy.
  DVE tensor_reduce: 1.04 ns/input-elem + 150ns, ANY dtype (no fast mode).
  DVE tensor_tensor max: 0.52 ns/output-elem + 150ns for packed 2-byte
    dtypes (2x_1p mode); pool/scalar engines cannot run tensor ops on TRN2.
Block maxima = fold tree (pairwise tensor_max halving block width 4x) + one
small reduce: 5.3us per 2MB chunk vs 9.1us for direct reduces.

Layout: head slice [2048, 4096] fp16 viewed as x[p, k] = [128, 65536]
(partition p holds rows 16p..16p+15 contiguously; k = t*4096 + c). Chunks
are arbitrary [c0, c0+clen) column ranges - no tile boundaries - and the
block-max output bm[p, t*32+b] reshapes straight to row-major [2048, 32].
The full 16MB fits in SBUF (128KB/partition), so every DMA job owns a
dedicated region: no write-after-read hazards anywhere.

Schedule: ring sizes/order chosen by an offline search against the
calibrated two-ring rate model so chunk completions (ring-FIFO) match the
vector engine's consumption order with minimal starvation under +-10% ring
rate skew. Small chunks first (vector starts ~10.5us), one big final chunk
(vector is backlogged at the end anyway; big chunks cost least per element).
"""

import numpy as np

H = 8
TGT = 2048
SRC = 4096
P = 128            # SBUF partitions
RPP = 16           # rows per partition
W = RPP * SRC      # 65536 elements per partition
NBLK = 32          # block maxima per original row
BLK = SRC // NBLK  # 128
NBLK_ALL = W // BLK  # 512 block maxima per partition
K_THRESH = 4
SAMPLED_T0 = 3

# (ring, clen) in VECTOR (predicted-arrival) order; c0 = running sum.
# 8 equal 2MB chunks (16KB descriptors on both rings -> even round-robin
# split); ring B's first issue is semaphore-gated on A0's completion so ring
# A streams chunk 0 solo (~full rate) and the rings stay offset by half a
# chunk for the rest of the stream (no lumpy pair completions).
SCHEDULE = [
    ("A", 8192), ("A", 8192), ("B", 8192), ("A", 8192),
    ("B", 8192), ("A", 8192), ("B", 8192), ("B", 8192),
]
GATE_B_ON = 0      # ring B's first dma_start waits for this job's s_in
# fold depth per chunk size (then one reduce over the remaining width)
FOLDK = {16384: 4, 8192: 4, 4096: 3, 2048: 2, 1024: 1}

JOBS = []          # (ring, c0, clen) in vector order
_g = 0
for _ring, _clen in SCHEDULE:
    JOBS.append((_ring, _g, _clen))
    _g += _clen
assert _g == W
NJOBS = len(JOBS)
FINAL_C0 = JOBS[-1][1]          # bulk store covers blocks [0, FINAL_C0/BLK)

_cache = {}


def _build_nc():
    """Raw Bass program, one head per core."""
    from contextlib import ExitStack

    import concourse.bass as bass
    import concourse.mybir as mybir

    nc = bass.Bass()
    f16 = mybir.dt.float16
    x = nc.declare_dram_parameter("x", [P, W], f16, isOutput=False)
    # bm[p, t*NBLK + b] = max of block b of row 16p + t
    bm = nc.declare_dram_parameter("bm", [P, NBLK_ALL], f16, isOutput=True)

    with ExitStack() as ctx:
        tiles = ctx.enter_context(nc.sbuf_tensor([P, W], f16))
        scra = ctx.enter_context(nc.sbuf_tensor([P, 8192], f16))
        scrb = ctx.enter_context(nc.sbuf_tensor([P, 4096], f16))
        bmsb = ctx.enter_context(nc.sbuf_tensor([P, NBLK_ALL], f16))
        s_in = [ctx.enter_context(nc.semaphore(f"s_in{j}")) for j in range(NJOBS)]
        s_red = ctx.enter_context(nc.semaphore("s_red"))
        s_out = ctx.enter_context(nc.semaphore("s_out"))
        block = ctx.enter_context(nc.Block())

        def issue_loads(eng, ring):
            first = True
            for j, (r, c0, clen) in enumerate(JOBS):
                if r != ring:
                    continue
                if ring == "B" and first:
                    eng.wait_ge(s_in[GATE_B_ON], 16)
                    first = False
                eng.dma_start(
                    out=tiles[:, c0 : c0 + clen],
                    in_=x[:, c0 : c0 + clen],
                ).then_inc(s_in[j], 16)

        @block.sync
        def _(sync):
            issue_loads(sync, "A")
            sync.wait_ge(s_out, 32)

        @block.scalar
        def _(scalar):
            issue_loads(scalar, "B")

        @block.gpsimd
        def _(gpsimd):
            # bulk store overlaps the final chunk's fold train
            gpsimd.wait_ge(s_red, NJOBS - 1)
            gpsimd.dma_start(
                out=bm[:, : FINAL_C0 // BLK], in_=bmsb[:, : FINAL_C0 // BLK]
            ).then_inc(s_out, 16)
            gpsimd.wait_ge(s_red, NJOBS)
            gpsimd.dma_start(
                out=bm[:, FINAL_C0 // BLK :], in_=bmsb[:, FINAL_C0 // BLK :]
            ).then_inc(s_out, 16)

        @block.vector
        def _(vector):
            for j, (r, c0, clen) in enumerate(JOBS):
                vector.wait_ge(s_in[j], 16)
                nblk = clen // BLK
                # fold tree: halve block width FOLDK times, ping-pong scratch
                src = tiles[:, c0 : c0 + clen]
                w = BLK
                for step in range(FOLDK[clen]):
                    dst = (scra if step % 2 == 0 else scrb)[:, : nblk * w // 2]
                    sv = src.rearrange("p (b c) -> p b c", c=w)
                    nc.vector.tensor_max(
                        out=dst.rearrange("p (b c) -> p b c", c=w // 2),
                        in0=sv[:, :, : w // 2],
                        in1=sv[:, :, w // 2 :],
                    )
                    src, w = dst, w // 2
                # finish with one reduce over the remaining width
                nc.vector.reduce_max(
                    out=bmsb[:, c0 // BLK : (c0 + clen) // BLK],
                    in_=src.rearrange("p (b c) -> p b c", c=w),
                    axis=mybir.AxisListType.X,
                ).then_inc(s_red, 1)

    return nc


def _get_nc():
    if "nc" not in _cache:
        _cache["nc"] = _build_nc()
    return _cache["nc"]


def run_device(aw, **run_kwargs):
    """Run the per-head fp16 block-max kernel on 8 cores.

    Takes the full f32 [H, TGT, SRC] tensor; returns ([H, TGT, NBLK] fp16
    block maxima of the fp16-cast data, results).
    """
    from concourse.bass_utils import run_bass_kernel_spmd

    nc = _get_nc()
    aw16 = aw.astype(np.float16)
    in_maps = [
        {"x": np.ascontiguousarray(aw16[c]).reshape(P, W)} for c in range(H)
    ]
    res = run_bass_kernel_spmd(nc, in_maps, list(range(H)), **run_kwargs)
    # bm[p, t*NBLK+b] -> row-major [TGT, NBLK]: row = 16p + t
    bms = [res.results[c]["bm"].reshape(TGT, NBLK) for c in range(H)]
    return np.stack(bms), res


def _exact_argmax(aw, bm):
    """Exact first-occurrence np.argmax(aw, -1) from fp16 block maxima.

    fp16 rounding is monotone, so every element equal to the f32 row max
    lives in a block whose fp16 block max ties the row's fp16 max. Scanning
    the tied blocks in ascending order preserves first-occurrence order.
    """
    rowmax = bm.max(-1, keepdims=True)
    mask = bm == rowmax  # [H, TGT, NBLK] candidate blocks
    cmax = int(mask.sum(-1).max())
    # candidate block indices in ascending order, padded with non-candidates
    order = np.argsort(~mask, axis=-1, kind="stable")[..., :cmax]
    valid = np.take_along_axis(mask, order, -1)
    blocks = aw.reshape(H, TGT, NBLK, BLK)
    win = np.take_along_axis(blocks, order[..., None], axis=2)  # [H,T,cmax,BLK]
    win = np.where(valid[..., None], win, -np.inf).reshape(H, TGT, cmax * BLK)
    j = win.argmax(-1)
    b = np.take_along_axis(order, (j // BLK)[..., None], -1)[..., 0]
    return b * BLK + j % BLK


def kernel(attention_weight):
    aw = np.asarray(attention_weight)
    assert aw.shape == (H, TGT, SRC), aw.shape
    aw = aw.astype(np.float32, copy=False)

    try:
        bm, _ = run_device(aw)
    except Exception as e:  # device path failed: fall back to host blockmax
        import traceback

        traceback.print_exc()
        print(f"WARNING: device path failed ({e!r}); falling back to numpy")
        bm = aw.astype(np.float16).reshape(H, TGT, NBLK, BLK).max(-1)

    cand = _exact_argmax(aw, bm)  # [H, TGT]
    present = np.zeros((H, SRC), np.float32)
    present[np.arange(H)[:, None], cand] = 1.0
    counting = present.sum(axis=0)

    if counting.max() <= K_THRESH:
        return np.broadcast_to(aw[SAMPLED_T0], aw.shape).copy()
    return aw
